# revision 41
# baseline (speedup 1.0000x reference)
"""Trainium2 Bass kernel for nn_BCA_17274358465235.

Module: out = x + conv1x1_up( softmax(fx @ fy_up^T) @ fself ) with
fx/fself = 2-layer 1x1-conv projections of x, fy = projection of
bilinearly-upsampled y.  B=4, CX=256, CY=512, CM=64, H=W=64 (N=4096
tokens), HY=WY=32.

Sharding: 8 cores = batch(4) x query-row-half(2).  Each core holds all
4096 keys (fy/fself replicated per batch) and 2048 query rows.  No
collectives; one SPMD program for all cores.

Per-core algorithm (layouts chosen so no transposes are needed):
  fself^T[key, c]  via second projection layer emitted transposed
  sim^T[key, row] = fy_f[:, keys]^T @ fx[:, rows]   (fp16 matmuls, two
      key-chunks ping-ponged across PE row-groups)
  exp: split between ACT (exact, bf16 out) and DVE (Schraudolph:
      int16(A*sim+B) bit-cast as bf16)
  fout^T[c, row] += fself^T_chunk^T @ exp_chunk   (PSUM accumulation,
      ones-column in fself^T produces the softmax denominator Z free)
  out = x + W_up @ (fout^T * (1/Z)) + b_up   (b_up via ones-row in W_up)

Key performance decisions (measured on hw; ~5us run-to-run noise from
the free-running HAM window / power state, so deltas under ~3us are
hard to confirm):
  - fp16 end-to-end for x/y/weights/projections; fp16 streams 1
    cycle/col on the PE and halves input DMA bytes.
  - exp split ACT/DVE per key-chunk (Schraudolph on DVE via
    tensor_scalar f32->int16 bit-cast as bf16).  Splitting each TILE
    across both engines measured WORSE (couples both queues to every
    unit).  End-to-end rel err ~1.45e-2 vs the 2e-2 gate.
  - the fy bias by2 is dropped: softmax cancels it exactly.
  - emission ORDER is load-bearing: tile lowers cross-engine deps to
    monotonic per-engine op-count waits; everything the first sims
    need (fx2, fy2 band 0) is emitted before later ACT/DVE work.  The
    attention-loop start is gated by pre-loop ACT/DVE queue DRAIN
    (not data arrival), so slack work (u2 tail, fs chunks 10+) hides
    in half-0's loop hooks -- but PE-queue work must stay pre-loop
    (a hook matmul waiting on a late DMA stalls the in-order PE queue
    and everything behind it).
  - fx2 needs NO row-group duplication: sims read fx2[0:64] only at
    cols {0:512, 1024:1536} and fx2[64:128] at {512:1024, 1536:2048},
    so fx blocks 1/3 write their PSUM at partitions 64-127 directly
    (column-group-offset matmul; bias column host-duplicated to rows
    64-127 for the lane-aligned add).  fy2's dup stays on ACT --
    SBUF->SBUF DMA dup measured slower (seam is latency-bound and the
    DMA adds sem+queue+transfer latency).
  - input DMA: all queues share 16 engines (~50-120GB/s effective at
    kernel start, descriptor-bound); baseline split (xs 0-1 on the
    scalar queue, rest ordered on sync) measured best.  yb-first and
    one-queue-strict-priority variants were slower.
  - fselfT PSUM->SBUF casts batched in pairs (one DVE CAST per 2 key
    chunks).
  - f16 output (host upcasts), one DMA per (quarter, ch-half), last
    quarter issued from the scalar queue.
  - keep-alive matmul gated on invz in half-1's pre-tail: holds the
    PE HAM clock at full rate across the last-pv -> up-projection gap.
  - no PE warm-up matmuls: the PE is power duty-cycle capped, dummy
    matmuls cost real budget.
  - TRIED AND REVERTED (all measured slower): column-group-packed pv
    pairs + 4-way-packed M=1 Z matmuls (correct, -256 PE cyc/unit,
    but the loop became latency-coupled through sim-PSUM buffer
    recycling: cadence 923 -> 1026ns/unit); per-tile dual-engine exp
    split; gpsimd tensor ops (10x slower than ACT, stall DVE via the
    shared SBUF port); SBUF->SBUF DMA for row-group dups.
"""
import sys

for _p in ("/opt/pypackages", "/opt/trn_rl_repo"):
    if _p not in sys.path:
        sys.path.insert(0, _p)

import numpy as np

import concourse.bacc as bacc
import concourse.mybir as mybir
import concourse.tile as tile
from concourse.bass_utils import run_bass_kernel_spmd

F32 = mybir.dt.float32
F32R = mybir.dt.float32r
F16 = mybir.dt.float16
BF16 = mybir.dt.bfloat16
I16 = mybir.dt.int16
EXP = mybir.ActivationFunctionType.Exp
COPY = mybir.ActivationFunctionType.Copy
IDENT = mybir.ActivationFunctionType.Identity
MUL = mybir.AluOpType.mult
ADD = mybir.AluOpType.add

B, CX, CY, CM = 4, 256, 512, 64
H = W = 64
HY = WY = 32
N = H * W              # 4096 tokens
NH = N // 2            # 2048 query rows per core
NYC = HY * WY          # 1024 coarse tokens
KC = N // 128          # 32 key chunks

# Schraudolph exp in bf16 bit-domain: bf16_bits(e^x) ~ int16(A16*x + B16)
A16 = float((1 << 7) / np.log(2.0))
B16 = 16250.12

# Which iteration indices (0..31) of each half-loop run exp on DVE
# (Schraudolph); the rest run exact exp on ACT.
DVE_H0 = frozenset(j for j in range(KC) if j % 2 == 1 and 3 <= j < 30)
DVE_H1 = frozenset(j for j in range(KC) if j % 2 == 1 and 5 <= j < 32)

_CACHE = {}


def _build():
    nc = bacc.Bacc("TRN2", target_bir_lowering=False, debug=False,
                   enable_asserts=False)

    # ---- DRAM I/O (per-core layouts pre-arranged on host) ----
    xs = nc.dram_tensor("xs", [128, 8192], F16, kind="ExternalInput").ap()
    xl = nc.dram_tensor("xl", [128, 4096], F16, kind="ExternalInput").ap()
    yb = nc.dram_tensor("yb", [128, 4096], F16, kind="ExternalInput").ap()
    wpack = nc.dram_tensor("wpack", [128, 1093], F16, kind="ExternalInput").ap()
    bias32 = nc.dram_tensor("bias32", [128, 3], F32, kind="ExternalInput").ap()
    ones = nc.dram_tensor("ones", [1, 4096], F16, kind="ExternalInput").ap()
    out = nc.dram_tensor("out", [128, 4096], F16, kind="ExternalOutput").ap()

    with tile.TileContext(nc) as tc:
        with tc.tile_pool(name="sbW", bufs=1) as sbW, \
             tc.tile_pool(name="sbM", bufs=1) as sbM:
            # ---- long-lived SBUF ----
            t_xs = sbM.tile([128, 8192], F16)      # full x for fself stream
            t_xl = sbM.tile([128, 4096], F16)      # fx input + residual
            fy2 = sbM.tile([128, 4096], F16)       # upsampled fy, duplicated
            fx2 = sbM.tile([128, 2048], F16)       # fx, duplicated
            fselfT = sbM.tile([128, 65 * KC], BF16)
            h1s_aug = sbM.tile([65, 4096], F16)    # W_self1 @ x with ones row
            scaled = sbM.tile([65, 2048], F16)     # [Z/Z; fout/Z] per row
            t_bias = sbM.tile([128, 3], F32)       # bx2 (rows 0-63 = 64-127)

            # ---- weights (single packed blob) ----
            t_wpack = sbW.tile([128, 1093], F16)
            t_ws1t = t_wpack[:, 0:128]
            t_ws2a = t_wpack[0:65, 128:194]
            t_wx1t = t_wpack[:, 194:322]
            t_wx2t = t_wpack[0:64, 322:386]
            t_wy1t = t_wpack[:, 387:643]
            t_wy2t = t_wpack[0:64, 643:707]
            t_wupt = t_wpack[0:65, 709:965]

            # ================= phase 1: projections =================
            with tc.tile_pool(name="sbP", bufs=1) as sbP, \
                 tc.tile_pool(name="psP1", bufs=1, space="PSUM") as psP1:
                t_yb = sbP.tile([128, 4096], F16)
                # input DMAs, critical-path first (baseline ordering:
                # measured better than yb-first or single-queue
                # variants -- per-queue descriptor throughput caps at
                # ~250GB/s, so xs 0-1 ride the scalar queue).
                nc.sync.dma_start(t_wpack[:, 0:709], wpack[:, 0:709])
                nc.sync.dma_start(t_bias[:], bias32[:])
                nc.sync.dma_start(h1s_aug[64:65, :], ones[:, 0:4096])
                for g in range(2):
                    nc.scalar.dma_start(t_xs[:, g * 1024:(g + 1) * 1024],
                                        xs[:, g * 1024:(g + 1) * 1024])
                nc.sync.dma_start(t_yb[:, 0:2048], yb[:, 0:2048])
                nc.sync.dma_start(t_yb[:, 2048:4096], yb[:, 2048:4096])
                for c in range(2):
                    nc.sync.dma_start(t_xl[:, c * 2048:(c + 1) * 2048],
                                      xl[:, c * 2048:(c + 1) * 2048])
                nc.sync.dma_start(t_wpack[:, 709:1093], wpack[:, 709:1093])
                for g in range(2, 8):
                    nc.sync.dma_start(t_xs[:, g * 1024:(g + 1) * 1024],
                                      xs[:, g * 1024:(g + 1) * 1024])

                def h1s_block(pool, idx, tag="blk", bufs=4):
                    p = pool.tile([64, 512], F32, tag=tag, bufs=bufs,
                                  name=f"pp_h1s_{idx}")
                    for a in range(2):
                        nc.tensor.matmul(p[:], t_ws1t[:, a * 64:(a + 1) * 64],
                                         t_xs[:, idx * 1024 + a * 512:
                                              idx * 1024 + a * 512 + 512],
                                         start=(a == 0), stop=(a == 1))
                    nc.vector.tensor_copy(
                        h1s_aug[0:64, idx * 512:idx * 512 + 512], p[:])

                def fs_pair(pool, j, tag="blk", bufs=4):
                    # two fs matmuls into one PSUM tile + ONE batched
                    # DVE cast for both chunks (strided 3D AP skips the
                    # 66th column of each).
                    p2 = pool.tile([128, 132], F32, tag=tag, bufs=bufs,
                                   name=f"pp_fs_{j}")
                    for k in range(2):
                        nc.tensor.matmul(p2[:, k * 66:(k + 1) * 66],
                                         h1s_aug[:, (j + k) * 128:(j + k + 1) * 128],
                                         t_ws2a, start=True, stop=True)
                    src = p2[:].rearrange("p (two c) -> p two c", two=2)
                    dst = fselfT[:, j * 65:(j + 2) * 65].rearrange(
                        "p (two c) -> p two c", two=2)
                    nc.vector.tensor_copy(dst[:, :, 0:65], src[:, :, 0:65])

                # fself preamble first: warms the PE HAM clock gate with
                # real work depending only on the first DMAs.  Only
                # chunks 0-5 (blocks 0-1) are produced pre-loop; the
                # rest streams through half-0's hook so the pre-loop
                # DVE/ACT queues stay short (the loop start is gated by
                # queue drain, not data arrival).
                h1s_block(psP1, 0)
                h1s_block(psP1, 1)
                fs_pair(psP1, 0)

                # warm the ACT exp table early
                t_dum = sbP.tile([1, 32], F32)
                nc.vector.memset(t_dum[:], 0.0)
                t_dum2 = sbP.tile([1, 32], F32)
                nc.scalar.activation(t_dum2[:], t_dum[:], EXP)

                # ---- fy path: h1y = Wy1 @ y ; fyc = Wy2 @ h1y ----
                # (fy bias by2 dropped: softmax cancels it exactly.)
                h1y_s = sbP.tile([64, 1024], F16)
                t2c = sbP.tile([64, 1024], F32)
                for blk in range(2):
                    p = psP1.tile([64, 512], F32, tag="blk", bufs=4,
                                  name=f"p_h1y_{blk}")
                    for a in range(4):
                        nc.tensor.matmul(
                            p[:], t_wy1t[:, a * 64:(a + 1) * 64],
                            t_yb[:, blk * 2048 + a * 512:blk * 2048 + a * 512 + 512],
                            start=(a == 0), stop=(a == 3))
                    nc.scalar.activation(h1y_s[:, blk * 512:blk * 512 + 512],
                                         p[:], COPY)
                p_fyc = psP1.tile([64, 1024], F32, tag="fyc", bufs=1,
                                  name="p_fyc")
                for blk in range(2):
                    nc.tensor.matmul(p_fyc[:, blk * 512:blk * 512 + 512],
                                     t_wy2t,
                                     h1y_s[:, blk * 512:blk * 512 + 512],
                                     start=True, stop=True)
                    nc.scalar.activation(
                        t2c[:, blk * 512:blk * 512 + 512],
                        p_fyc[:, blk * 512:blk * 512 + 512], COPY, scale=0.25)

                # H pass, ops 1-3 first (image rows 0..31) so W-pass
                # band 0 isn't stuck behind the whole H pass on DVE.
                fyH = sbM.tile([64, 2048], F32)
                t1v = p_fyc[:].rearrange("p (h w) -> p h w", h=32)
                t2v = t2c[:].rearrange("p (h w) -> p h w", h=32)
                fe = fyH[:].rearrange("p (h two w) -> p h two w", h=32, two=2)
                HSTT = nc.vector.scalar_tensor_tensor
                HSTT(fe[:, 0, 0, :], t1v[:, 0, :], 0.75, t2v[:, 0, :], MUL, ADD)
                HSTT(fe[:, 1:16, 0, :], t1v[:, 1:16, :], 0.75, t2v[:, 0:15, :], MUL, ADD)
                HSTT(fe[:, 0:15, 1, :], t1v[:, 0:15, :], 0.75, t2v[:, 1:16, :], MUL, ADD)

                def hpass_tail():
                    HSTT(fe[:, 16:32, 0, :], t1v[:, 16:32, :], 0.75,
                         t2v[:, 15:31, :], MUL, ADD)
                    HSTT(fe[:, 15:31, 1, :], t1v[:, 15:31, :], 0.75,
                         t2v[:, 16:32, :], MUL, ADD)
                    HSTT(fe[:, 31, 1, :], t1v[:, 31, :], 0.75,
                         t2v[:, 31, :], MUL, ADD)

                # 0.25-scaled fyH copy (gpsimd tensor ops measured 10x
                # slower than ACT and stall DVE via the shared SBUF
                # port -- keep this on ACT)
                u2 = sbM.tile([64, 2048], F32)
                u2v = u2[:].rearrange("p (h w) -> p h w", h=64)
                fyHv = fyH[:].rearrange("p (h w) -> p h w", h=64)
                nc.scalar.activation(u2[:, 0:31 * 32], fyH[:, 0:31 * 32],
                                     COPY, scale=0.25)

                # W pass + row-group duplication in 4 h-bands.  The
                # row-group dup goes through SBUF->SBUF DMA (sync
                # queue) instead of ACT.
                fw = fy2[0:64, :].rearrange("p (h w two) -> p h w two",
                                            h=64, two=2)
                _BANDS = ((slice(0, 16), 0, 1024),
                          (slice(16, 31), 1024, 1984),
                          (slice(31, 48), 1984, 3072),
                          (slice(48, 64), 3072, 4096))

                STT = nc.vector.scalar_tensor_tensor

                def wpass_band(b):
                    hs, c0, c1 = _BANDS[b]
                    nc.vector.tensor_copy(fw[:, hs, 0, 0], fyHv[:, hs, 0])
                    STT(fw[:, hs, 1:32, 0], fyHv[:, hs, 1:32], 0.75,
                        u2v[:, hs, 0:31], MUL, ADD)
                    STT(fw[:, hs, 0:31, 1], fyHv[:, hs, 0:31], 0.75,
                        u2v[:, hs, 1:32], MUL, ADD)
                    nc.vector.tensor_copy(fw[:, hs, 31, 1], fyHv[:, hs, 31])
                    # row-group dup on ACT (measured faster than a
                    # SBUF->SBUF DMA here: the seam is latency-bound
                    # and the DMA path adds sem+queue+transfer latency).
                    # Band 0's dup is split so the first sims' MM_b
                    # operand (chunks 0-3 = cols 0:512) lands earlier.
                    if b == 0:
                        nc.scalar.activation(fy2[64:128, c0:c0 + 512],
                                             fy2[0:64, c0:c0 + 512], COPY)
                        nc.scalar.activation(fy2[64:128, c0 + 512:c1],
                                             fy2[0:64, c0 + 512:c1], COPY)
                    else:
                        nc.scalar.activation(fy2[64:128, c0:c1],
                                             fy2[0:64, c0:c1], COPY)

                # band 0 immediately -- before the fx path -- so the
                # first sims' waits don't extend past the fx chain.
                wpass_band(0)

                # ---- fx path: h1x = Wx1 @ xl ; fx = Wx2 @ h1x + bx2 ----
                # Emitted BEFORE hpass_tail: the first sims wait on fx2
                # and must not queue behind the rest of the fy chain.
                # NO duplication: the sims only read fx2[0:64] at cols
                # {0:512, 1024:1536} and fx2[64:128] at cols {512:1024,
                # 1536:2048}, so blocks 1/3 write their PSUM at
                # partitions 64-127 directly (col-group-offset matmul)
                # and the bias add stays lane-aligned via a host-
                # duplicated bias column.
                h1x_s = sbP.tile([64, 2048], F16)
                for blk in range(4):
                    p = psP1.tile([64, 512], F32, tag="blk", bufs=4,
                                  name=f"p_h1x_{blk}")
                    for a in range(2):
                        nc.tensor.matmul(
                            p[:], t_wx1t[:, a * 64:(a + 1) * 64],
                            t_xl[:, blk * 1024 + a * 512:blk * 1024 + a * 512 + 512],
                            start=(a == 0), stop=(a == 1))
                    nc.scalar.activation(h1x_s[:, blk * 512:blk * 512 + 512],
                                         p[:], COPY)
                for blk in range(4):
                    rg = slice(0, 64) if blk % 2 == 0 else slice(64, 128)
                    p = psP1.tile([128, 512], F32, tag="blk", bufs=4,
                                  name=f"p_fx_{blk}")
                    nc.tensor.matmul(p[rg, :], t_wx2t,
                                     h1x_s[:, blk * 512:blk * 512 + 512],
                                     start=True, stop=True)
                    nc.vector.tensor_scalar_add(
                        fx2[rg, blk * 512:blk * 512 + 512], p[rg, :],
                        t_bias[rg, 0:1])

                # rest of the fy H pass (consumed by W-pass band 1 in
                # half-0's first hook)
                hpass_tail()

                # more of the fself stream while the fy chain finishes;
                # blocks 6-7 stay in the half-0 hook (their xs DMAs
                # land last).  These matmuls fill the PE while it waits
                # for the fy/fx chains -- they must stay PRE-loop: a
                # hook matmul waiting on a late xs DMA stalls the
                # in-order PE queue and blocks the attention matmuls
                # behind it.
                for idx in range(2, 6):
                    h1s_block(psP1, idx)
                    fs_pair(psP1, 2 * idx - 2)

            # ================= phase 2: attention =================
            fout_accs = {}

            def sim_unit(pool, j, h):
                ps = pool.tile([128, 1024], F32, tag="sim", bufs=2,
                               name=f"sim_{j}_{h}")
                nc.tensor.matmul(
                    ps[:, 0:512], fy2[0:64, j * 128:(j + 1) * 128],
                    fx2[0:64, h * 1024:h * 1024 + 512],
                    start=True, stop=True)
                nc.tensor.matmul(
                    ps[:, 512:1024], fy2[64:128, j * 128:(j + 1) * 128],
                    fx2[64:128, h * 1024 + 512:h * 1024 + 1024],
                    start=True, stop=True)
                return ps

            def exp_unit(st, j, h, on_dve):
                et = sbM.tile([128, 1024], BF16, tag="et", bufs=8,
                              name=f"et_{j}_{h}")
                if j == KC - 1:
                    # last chunk: split across BOTH engines to halve
                    # the exp latency on the half-loop's serial ending
                    # (exp31 -> pv31 -> pre-tail chain).  Per-unit
                    # splitting everywhere measured worse (couples both
                    # queues to every unit), but the final tile has no
                    # pipeline behind it.
                    nc.scalar.activation(et[:, 0:512], st[:, 0:512], EXP)
                    nc.vector.tensor_scalar(et[:, 512:1024].bitcast(I16),
                                            st[:, 512:1024],
                                            A16, B16, MUL, ADD)
                elif on_dve:
                    nc.vector.tensor_scalar(et[:].bitcast(I16), st[:],
                                            A16, B16, MUL, ADD)
                else:
                    nc.scalar.activation(et[:], st[:], EXP)
                return et

            def pv_unit(fout_acc, et, j):
                w = fselfT[:, j * 65:(j + 1) * 65]
                nc.tensor.matmul(fout_acc[:, 0:512], w, et[:, 0:512],
                                 start=(j == 0), stop=(j == KC - 1))
                nc.tensor.matmul(fout_acc[:, 512:1024], w, et[:, 512:1024],
                                 start=(j == 0), stop=(j == KC - 1))

            def half_loop(h, psB, hook, dve_set):
                fout_acc = fout_accs[h]
                sims = {}
                sims[0] = sim_unit(psB, 0, h)
                sims[1] = sim_unit(psB, 1, h)
                sims[2] = sim_unit(psB, 2, h)
                for j in range(KC):
                    if hook is not None:
                        hook(j)
                    et = exp_unit(sims.pop(j), j, h, j in dve_set)
                    pv_unit(fout_acc, et, j)
                    if j + 3 < KC:
                        sims[j + 3] = sim_unit(psB, j + 3, h)

            def pre_tail(h, keepalive=None):
                # 1/Z scaling of fout into `scaled`
                fout_acc = fout_accs[h]
                invzbs = []
                for s in range(2):
                    cs = slice(s * 512, (s + 1) * 512)
                    invz = sbM.tile([1, 512], F32, tag="zrow", bufs=2,
                                    name=f"invz_{h}_{s}")
                    nc.vector.reciprocal_approx_fast(invz[:], fout_acc[0:1, cs])
                    if s == 0 and keepalive is not None:
                        # tiny matmul gated on invz: keeps the PE HAM
                        # clock warm across the otherwise-idle stretch
                        # between the last pv and the up-projections
                        # (a >3.4us gap re-throttles the PE to 1.2GHz)
                        kp = keepalive.tile([128, 1024], F32, tag="sim",
                                            bufs=2, name="ka")
                        nc.tensor.matmul(kp[0:1, 0:64],
                                         t_wpack[0:1, 0:1],
                                         invz[:].bitcast(F16)[:, 0:64],
                                         start=True, stop=True)
                    invzb = sbM.tile([128, 512], F32, tag="izb", bufs=2,
                                     name=f"invzb_{h}_{s}")
                    nc.gpsimd.partition_broadcast(invzb[:], invz[:])
                    invzbs.append(invzb)
                for s in range(2):
                    cs = slice(s * 512, (s + 1) * 512)
                    nc.vector.tensor_mul(
                        scaled[:, h * 1024 + s * 512:h * 1024 + (s + 1) * 512],
                        fout_acc[:, cs], invzbs[s][0:65, :])

            def up_quarter(psC, q, dma_eng=None):
                # up-projection + residual + output DMA for 512 query px
                eng = dma_eng or nc.sync
                for a in range(2):
                    p = psC.tile([128, 512], F32, tag="up", bufs=2,
                                 name=f"p_up_{q}_{a}")
                    nc.tensor.matmul(p[:], t_wupt[:, a * 128:(a + 1) * 128],
                                     scaled[:, q * 512:(q + 1) * 512],
                                     start=True, stop=True)
                    out_s = sbM.tile([128, 512], F16, tag="tail", bufs=4,
                                     name=f"out_s_{q}_{a}")
                    xv = t_xl[:, q * 1024 + a * 512:
                              q * 1024 + a * 512 + 512]
                    nc.vector.tensor_add(out_s[:], p[:], xv)
                    eng.dma_start(
                        out[:, a * 2048 + q * 512:a * 2048 + (q + 1) * 512],
                        out_s[:])

            with tc.tile_pool(name="psA0", bufs=1, space="PSUM") as psA0:
                fout_accs[0] = psA0.tile([65, 1024], F32, name="fout0")
                with tc.tile_pool(name="psFS", bufs=1, space="PSUM") as psFS:

                    def hook0(j):
                        if j == 0:
                            wpass_band(1)
                        elif j == 1:
                            # u2 tail, consumed by bands 2-3
                            nc.scalar.activation(u2[:, 31 * 32:2048],
                                                 fyH[:, 31 * 32:2048],
                                                 COPY, scale=0.25)
                        elif j == 4:
                            wpass_band(2)
                        elif j == 10:
                            wpass_band(3)
                        if j < 2:
                            h1s_block(psFS, 6 + j, tag="fs", bufs=2)
                        if 10 + 2 * j < KC:
                            fs_pair(psFS, 10 + 2 * j, tag="fs", bufs=2)

                    with tc.tile_pool(name="psB0", bufs=1,
                                      space="PSUM") as psB0:
                        half_loop(0, psB0, hook0, DVE_H0)

                # NOTE: pre_tail(0) must stay OUTSIDE psB0: a keepalive
                # tile inside would hold the sim banks through the
                # recip->bcast->mul chain and delay half-1's sims
                # (measured +3us)
                pre_tail(0)

            with tc.tile_pool(name="psA1", bufs=1, space="PSUM") as psA1:
                fout_accs[1] = psA1.tile([65, 1024], F32, name="fout1")
                with tc.tile_pool(name="psC", bufs=1, space="PSUM") as psC:

                    def hook1(j):
                        # half-0 tail interleaved into half-1's loop
                        if j == 6:
                            up_quarter(psC, 0)
                        elif j == 12:
                            up_quarter(psC, 1)

                    with tc.tile_pool(name="psB1", bufs=1,
                                      space="PSUM") as psB1:
                        half_loop(1, psB1, hook1, DVE_H1)
                        pre_tail(1, psB1)
                    up_quarter(psC, 2)
                    up_quarter(psC, 3, dma_eng=nc.scalar)

    nc.compile()
    return nc


def _prep_maps(x, y, W_self1, b_self1, W_self2, b_self2, W_x1, b_x1, W_x2,
               b_x2, W_y1, b_y1, W_y2, b_y2, W_up, b_up):
    f64 = np.float64

    def fold(W2, b1, b2):
        return (W2.astype(f64) @ b1.astype(f64) + b2.astype(f64)).astype(np.float32)

    ws2a = np.zeros((65, 66), np.float16)
    ws2a[64, 0] = 1.0
    ws2a[0:64, 1:65] = W_self2.T.astype(np.float16)
    ws2a[64, 1:65] = fold(W_self2, b_self1, b_self2).astype(np.float16)
    bx2 = fold(W_x2, b_x1, b_x2).reshape(64, 1)
    b3 = np.concatenate([bx2, np.zeros((64, 2))], axis=1).astype(np.float32)
    bias32 = np.ascontiguousarray(np.concatenate([b3, b3], axis=0))

    ws1t = np.ascontiguousarray(
        W_self1.T.reshape(2, 128, 64).transpose(1, 0, 2).reshape(128, 128))
    wx1t = np.ascontiguousarray(
        W_x1.T.reshape(2, 128, 64).transpose(1, 0, 2).reshape(128, 128))
    wy1t = np.ascontiguousarray(
        W_y1.T.reshape(4, 128, 64).transpose(1, 0, 2).reshape(128, 256))
    wx2t = np.ascontiguousarray(W_x2.T)
    wy2t = np.ascontiguousarray(W_y2.T)
    wupt = np.ascontiguousarray(
        np.concatenate([b_up.reshape(1, 256), W_up.T], axis=0))
    wp = np.zeros((128, 1093), np.float16)
    wp[:, 0:128] = ws1t.astype(np.float16)
    wp[0:65, 128:194] = ws2a
    wp[:, 194:322] = wx1t.astype(np.float16)
    wp[0:64, 322:386] = wx2t.astype(np.float16)
    wp[:, 387:643] = wy1t.astype(np.float16)
    wp[0:64, 643:707] = wy2t.astype(np.float16)
    wp[0:65, 709:965] = wupt.astype(np.float16)
    wp[0:1, 965:1093] = 1.0

    _ONES = np.ones((1, 4096), np.float16)
    maps = []
    for b in range(B):
        xf = x[b].reshape(CX, N).astype(np.float16)             # [256, 4096]
        xs_h = np.ascontiguousarray(
            xf.reshape(2, 128, 8, 512).transpose(1, 2, 0, 3).reshape(128, 8192))
        yf = y[b].reshape(CY, NYC).astype(np.float16)
        yb_h = np.ascontiguousarray(
            yf.reshape(4, 128, 2, 512).transpose(1, 2, 0, 3).reshape(128, 4096))
        for half in range(2):
            xh = xf[:, half * NH:(half + 1) * NH]               # [256, 2048]
            xl_h = np.ascontiguousarray(
                xh.reshape(2, 128, 4, 512).transpose(1, 2, 0, 3).reshape(128, 4096))
            maps.append({
                "xs": xs_h, "xl": xl_h, "yb": yb_h,
                "wpack": wp, "bias32": bias32, "ones": _ONES,
            })
    return maps


def _run(inputs, trace=False, trace_kwargs=None):
    if "nc" not in _CACHE:
        _CACHE["nc"] = _build()
    nc = _CACHE["nc"]
    maps = _prep_maps(**inputs)
    res = run_bass_kernel_spmd(nc, maps, list(range(8)), trace=trace,
                               **(trace_kwargs or {}))
    outs = np.empty((B, CX, H, W), np.float32)
    for b in range(B):
        for half in range(2):
            o = res.results[2 * b + half]["out"].astype(np.float32)
            oh = o.reshape(128, 2, NH).transpose(1, 0, 2).reshape(CX, NH)
            outs[b, :, :, :].reshape(CX, N)[:, half * NH:(half + 1) * NH] = oh
    return outs, res


def kernel(**inputs):
    outs, _ = _run(inputs, trace=False)
    return outs
